# revision 11
# baseline (speedup 1.0000x reference)
"""LayerNorm-LSTM (2-layer, shared h/c across layers, per-sample weights) on 8 TRN2 cores.

Strategy: data-parallel over batch (2 samples/core). Per core:
  Phase A: load inputs; Phase B: precompute layer-0 input projections
  gates_x[t] = Wih0 @ x_t + b0 for all t (batched matmul, N=256);
  Phase C: sequential recurrence over t.

Recurrence matvec runs col-tiled on the PE: 4 concurrent column-group
streams (tile_position=(0,32g)), stationary = h K-block [128,1], moving
= W^T chunk [128,512] fp32r. Gate chunk g lands on psum partition 32g;
rows {0,32,64,96} are evacuated to a zero-padded SBUF tile and
PE-transposed ([128,128] x 4 chunks) into column layout
gcol[p, g, cc] = gate g at position 4p+cc. All LN/elementwise work then
runs sample-batched at 128 partitions x small free dims. LN stats come
from a (1/512)-matrix partition-reduction matmul; rstd is computed with
a Quake-style rsqrt (bitcast seed + 1 Newton step) on DVE so the ACT
engine only ever uses the sigmoid/tanh/square/copy table set (zero
LoadActFuncSet reloads). No scatter DMAs in the loop.

Gate order is host-permuted from [i,f,g,o] to [i,f,o,g] so one sigmoid
covers gates 0..2 and one tanh covers gate 3.
"""

import sys

sys.path.insert(0, "/opt/trn_rl_repo")

import numpy as np

import concourse.bacc as bacc
import concourse.bass as bass
import concourse.tile as tile
from concourse import mybir
from concourse.bass_utils import run_bass_kernel_spmd

F32 = mybir.dt.float32
F32R = mybir.dt.float32r
I32 = mybir.dt.int32
AF = mybir.ActivationFunctionType
OP = mybir.AluOpType

B, S, D, H = 16, 256, 512, 512
NCORES = 8
BPC = B // NCORES  # samples per core
EPS = 1e-5
QUAKE_K = 0x5F3759DF
# permutation taking reference gate order [i,f,g,o] -> [i,f,o,g]
GATE_PERM = np.concatenate(
    [np.arange(0, 512), np.arange(512, 1024), np.arange(1536, 2048), np.arange(1024, 1536)]
)


def build_program(s_steps=S, apply_ln_affine=False):
    """Build the per-core SPMD Bass program. Returns the compiled Bacc."""
    nc = bacc.Bacc("TRN2", target_bir_lowering=False, debug=False, num_devices=NCORES)

    # ---- DRAM parameters (per-core shapes) ----
    # xT[s, k, d, t] = x[s, t, 4d+k]
    xT_d = nc.dram_tensor("xT", [BPC, 4, 128, s_steps], F32, kind="ExternalInput").ap()
    # w0T[s, m=(g,q), k, d, j] = W0_perm[512g + 4j + q, 4d + k]
    w0T_d = nc.dram_tensor("w0T", [BPC, 16, 4, 128, 128], F32, kind="ExternalInput").ap()
    # whT[s, l, k, d, n] = Wrec_perm[r(n), 4d + k]; col group g streams
    # n in [512g, 512g+512): streamed col j -> row 512g + 4*(j%128) + j//128
    whT_d = nc.dram_tensor("whT", [BPC, 2, 4, 128, 2048], F32R, kind="ExternalInput").ap()
    # bias cols: b0col[p, s, g*4+cc] = b0_perm[512g + 4p + cc]
    b0c_d = nc.dram_tensor("b0col", [128, BPC, 16], F32, kind="ExternalInput").ap()
    b1c_d = nc.dram_tensor("b1col", [128, BPC, 16], F32, kind="ExternalInput").ap()
    lnw_d = nc.dram_tensor("lnw", [128, 2, 16], F32, kind="ExternalInput").ap()
    lnb_d = nc.dram_tensor("lnb", [128, 2, 16], F32, kind="ExternalInput").ap()
    # output: ys[p, s, t, cc] = h1(t)[4p+cc] for sample s
    ys_d = nc.dram_tensor("ys", [128, BPC, s_steps, 4], F32, kind="ExternalOutput").ap()

    from contextlib import ExitStack

    with tile.TileContext(nc) as tc, ExitStack() as ctx:
        consts = ctx.enter_context(tc.tile_pool(name="consts", bufs=1))
        wpool = ctx.enter_context(tc.tile_pool(name="weights", bufs=1))
        xpool = ctx.enter_context(tc.tile_pool(name="xproj", bufs=6))
        state = ctx.enter_context(tc.tile_pool(name="state", bufs=1))
        work = ctx.enter_context(tc.tile_pool(name="work", bufs=3))
        h0pool = ctx.enter_context(tc.tile_pool(name="h0", bufs=2))
        psg = ctx.enter_context(tc.tile_pool(name="psg", bufs=1, space="PSUM"))
        psT = ctx.enter_context(tc.tile_pool(name="psT", bufs=2, space="PSUM"))
        psum = ctx.enter_context(tc.tile_pool(name="psum", bufs=1, space="PSUM"))

        if True:
            # ---- constants / persistent tiles ----
            ident = consts.tile([128, 128], F32)
            nc.gpsimd.memset(ident, 0.0)
            from concourse.masks import make_identity

            make_identity(nc, ident, nomemset=True)

            # partition-reduction matrix pre-scaled by 1/512: stats matmuls
            # directly produce E[.]
            ones = consts.tile([128, 128], F32)
            nc.vector.memset(ones, 1.0 / 512.0)
            kc = consts.tile([128, 8], I32)
            nc.vector.memset(kc, QUAKE_K)
            one_i = consts.tile([128, 8], I32)
            nc.vector.memset(one_i, 1)
            hzero_f = consts.tile([128, 4], F32)
            nc.vector.memset(hzero_f, 0.0)
            hzero = consts.tile([128, 4], F32R)
            nc.vector.tensor_copy(hzero, hzero_f)

            b0col = consts.tile([128, BPC, 16], F32)
            nc.sync.dma_start(out=b0col, in_=b0c_d)
            b1col = consts.tile([128, BPC, 16], F32)
            nc.sync.dma_start(out=b1col, in_=b1c_d)
            if apply_ln_affine:
                lnw = consts.tile([128, 2, 16], F32)
                nc.sync.dma_start(out=lnw, in_=lnw_d)
                lnb = consts.tile([128, 2, 16], F32)
                nc.sync.dma_start(out=lnb, in_=lnb_d)

            # recurrent weights, SBUF-resident (16 MB), partition-first
            whT = wpool.tile([128, BPC, 2, 4, 2048], F32R)
            for s in range(BPC):
                for l in range(2):
                    for k in range(4):
                        nc.sync.dma_start(out=whT[:, s, l, k], in_=whT_d[s, l, k])

            # gates_x in col layout: gx[p, s, t, g*4+cc]
            gx = wpool.tile([128, BPC, s_steps, 16], F32)
            # xT resident (1 MB), partition-first
            xTs = wpool.tile([128, BPC, 4, s_steps], F32)
            for s in range(BPC):
                for k in range(4):
                    nc.sync.dma_start(out=xTs[:, s, k], in_=xT_d[s, k])

            # transpose staging; rows other than {0,32,64,96} carry zeros
            grow4 = []
            for s in range(BPC):
                g4 = state.tile([128, 512], F32, tag=f"grow4_{s}")
                nc.vector.memset(g4, 0.0)
                grow4.append(g4)

            # ---- Phase B: x-projection ----
            for s in range(BPC):
                for m in range(16):
                    pxa = psum.tile([128, s_steps], F32, tag="sums")
                    wt = []
                    for _k in range(4):
                        w0t = xpool.tile([128, 128], F32, tag="w0t")
                        wt.append(w0t)
                    for k in range(4):
                        nc.sync.dma_start(out=wt[k], in_=w0T_d[s, m, k])
                    for k in range(4):
                        nc.tensor.matmul(
                            pxa,
                            wt[k],
                            xTs[:, s, k],
                            start=(k == 0),
                            stop=(k == 3),
                        )
                    # gx[:, s, :, m] = pxa + b0col[:, s, m]
                    nc.vector.tensor_scalar(
                        gx[:, s, :, m : m + 1].rearrange("p t o -> p (t o)"),
                        pxa,
                        b0col[:, s, m : m + 1],
                        None,
                        OP.add,
                    )

            # ---- persistent recurrence state ----
            ys_sb = state.tile([128, BPC, s_steps, 4], F32R)
            # cstate[:, s, 0:4] = c, [:, s, 4:8] = c^2
            cst = state.tile([128, BPC, 8], F32)
            nc.vector.memset(cst, 0.0)

            def quake_rsqrt(v, yi, t_, r, n):
                """r = 1/sqrt(v + EPS), one Newton step. v,t_,r: F32 [128,n];
                yi: I32 [128,n]. Uses kc/one_i consts (width 8 >= n)."""
                nc.vector.tensor_scalar(v, v, EPS, None, OP.add)
                nc.vector.tensor_tensor(
                    yi, v.bitcast(I32), one_i[:, 0:n], OP.logical_shift_right
                )
                nc.vector.tensor_tensor(yi, kc[:, 0:n], yi, OP.subtract)
                y = yi.bitcast(F32)
                nc.vector.tensor_tensor(t_, y, y, OP.mult)
                nc.vector.tensor_tensor(t_, t_, v, OP.mult)
                nc.vector.tensor_scalar(t_, t_, -0.5, 1.5, OP.mult, OP.add)
                nc.vector.tensor_tensor(r, y, t_, OP.mult)

            # ---- Phase C: recurrence ----
            for t in range(s_steps):
                h0t = h0pool.tile([128, BPC, 4], F32R, tag="h0")
                for l in range(2):
                    gvs = {}
                    for s in range(BPC):
                        # --- matvec: gates = Wrec @ h, 4 col-group streams ---
                        if l == 0:
                            hin = hzero if t == 0 else ys_sb[:, s, t - 1]
                        else:
                            hin = h0t[:, s]
                        pgA = psg.tile([1, 1024], F32, tag="gA")
                        pgB = psg.tile([1, 1024], F32, tag="gB")
                        halves = (pgA, pgB)
                        # half A fully accumulated first so its evac+transpose
                        # overlaps half B's matmuls
                        for hf in range(2):
                            for k in range(4):
                                lhs = hin[:, k : k + 1].bitcast(F32R)
                                for j in range(2):
                                    g = 2 * hf + j
                                    nc.tensor.matmul(
                                        halves[hf][0:1, 512 * j : 512 * (j + 1)],
                                        lhs,
                                        whT[:, s, l, k, 512 * g : 512 * (g + 1)].bitcast(F32R),
                                        start=(k == 0),
                                        stop=(k == 3),
                                    )
                        # --- evac: cross-partition single-lane copies spread
                        # gate chunk g to staging row 32g (DVE+ACT in parallel)
                        for g in range(4):
                            src = halves[g // 2][0:1, 512 * (g % 2) : 512 * (g % 2 + 1)]
                            dst = grow4[s][32 * g : 32 * g + 1, :]
                            if g % 2 == 0:
                                nc.vector.tensor_copy(dst, src)
                            else:
                                nc.scalar.copy(dst, src)
                        # --- PE transpose to column layout ---
                        # gcolp[p, cc, 32g] = gate g at position 4p+cc
                        gcolp = psT.tile([128, 4, 128], F32, tag="gT")
                        for cc in range(4):
                            nc.tensor.transpose(
                                gcolp[:, cc, :],
                                grow4[s][:, 128 * cc : 128 * (cc + 1)],
                                ident,
                            )
                        gvs[s] = gcolp[:, :, 0:97:32].rearrange("p cc g -> p g cc")

                    # ---- batched LN + activations + state update ----
                    # combo free order: (s, half, g, cc), cc last for the fold
                    combo = work.tile([128, BPC, 2, 16], F32, tag="combo")
                    for s in range(BPC):
                        badd = gx[:, s, t] if l == 0 else b1col[:, s]
                        nc.vector.tensor_tensor(
                            combo[:, s, 0].rearrange("p (g cc) -> p g cc", g=4),
                            gvs[s],
                            badd.rearrange("p (g cc) -> p g cc", g=4),
                            OP.add,
                        )
                    nc.scalar.activation(
                        combo[:, :, 1], combo[:, :, 0], AF.Square
                    )
                    # --- LN stats: E[g], E[g^2] via (1/512)-matmul + cc-fold ---
                    psums = psum.tile([128, 64], F32, tag="sums")
                    nc.tensor.matmul(
                        psums,
                        ones,
                        combo.rearrange("p s h m -> p (s h m)"),
                        start=True,
                        stop=True,
                    )
                    St = work.tile([128, 16], F32, tag="St")  # (s, h, g)
                    nc.vector.tensor_reduce(
                        St,
                        psums.rearrange("p (s h g cc) -> p (s h g) cc", s=2, h=2, g=4),
                        mybir.AxisListType.X,
                        OP.add,
                    )
                    Stv = St.rearrange("p (s h g) -> p s h g", s=2, h=2)
                    mean = Stv[:, :, 0]  # [128, 2, 4]
                    ex2 = Stv[:, :, 1]
                    V = work.tile([128, 8], F32, tag="V")
                    Vv = V.rearrange("p (s g) -> p s g", s=2)
                    nc.vector.tensor_tensor(Vv, mean, mean, OP.mult)
                    nc.vector.tensor_tensor(Vv, ex2, Vv, OP.subtract)
                    Yi = work.tile([128, 8], I32, tag="Yi")
                    T = work.tile([128, 8], F32, tag="T")
                    R = work.tile([128, 8], F32, tag="R")
                    quake_rsqrt(V, Yi, T, R, 8)
                    Rv = R.rearrange("p (s g) -> p s g", s=2)
                    # --- normalize + (affine) + activations ---
                    wk = work.tile([128, BPC, 4, 4], F32, tag="wk")  # (s, g, cc)
                    nc.vector.tensor_tensor(
                        wk,
                        combo[:, :, 0].rearrange("p s (g cc) -> p s g cc", g=4),
                        mean[:, :, :, None].to_broadcast((128, 2, 4, 4)),
                        OP.subtract,
                    )
                    nc.vector.tensor_tensor(
                        wk,
                        wk,
                        Rv[:, :, :, None].to_broadcast((128, 2, 4, 4)),
                        OP.mult,
                    )
                    if apply_ln_affine:
                        lnwv = lnw[:, l].rearrange("p (g cc) -> p g cc", g=4)
                        lnbv = lnb[:, l].rearrange("p (g cc) -> p g cc", g=4)
                        nc.vector.tensor_tensor(
                            wk, wk, lnwv[:, None].to_broadcast((128, 2, 4, 4)), OP.mult
                        )
                        nc.vector.tensor_tensor(
                            wk, wk, lnbv[:, None].to_broadcast((128, 2, 4, 4)), OP.add
                        )
                    nc.scalar.activation(wk[:, :, 0:3], wk[:, :, 0:3], AF.Sigmoid)
                    nc.scalar.activation(wk[:, :, 3], wk[:, :, 3], AF.Tanh)
                    # --- c update: c = f*c + i*g ---
                    tmp = work.tile([128, BPC, 2, 4], F32, tag="tmp")
                    nc.vector.tensor_tensor(
                        tmp[:, :, 0], wk[:, :, 0], wk[:, :, 3], OP.mult
                    )  # i*g
                    cv = cst.rearrange("p s (h cc) -> p s h cc", h=2)
                    nc.vector.tensor_tensor(
                        tmp[:, :, 1], wk[:, :, 1], cv[:, :, 0], OP.mult
                    )  # f*c
                    nc.vector.tensor_tensor(
                        cv[:, :, 0], tmp[:, :, 0], tmp[:, :, 1], OP.add
                    )
                    nc.scalar.activation(cv[:, :, 1], cv[:, :, 0], AF.Square)
                    # --- LN(c) ---
                    pcs = psum.tile([128, 16], F32, tag="csums")
                    nc.tensor.matmul(
                        pcs,
                        ones,
                        cst.rearrange("p s m -> p (s m)"),
                        start=True,
                        stop=True,
                    )
                    CS = work.tile([128, 4], F32, tag="CS")  # (s, h)
                    nc.vector.tensor_reduce(
                        CS,
                        pcs.rearrange("p (s h cc) -> p (s h) cc", s=2, h=2),
                        mybir.AxisListType.X,
                        OP.add,
                    )
                    CSv = CS.rearrange("p (s h) -> p s h", s=2)
                    cmean = CSv[:, :, 0:1]  # [128, 2, 1]
                    cex2 = CSv[:, :, 1:2]
                    V2 = work.tile([128, 2], F32, tag="V2")
                    V2v = V2.rearrange("p (s o) -> p s o", s=2)
                    nc.vector.tensor_tensor(V2v, cmean, cmean, OP.mult)
                    nc.vector.tensor_tensor(V2v, cex2, V2v, OP.subtract)
                    Yi2 = work.tile([128, 2], I32, tag="Yi2")
                    T2 = work.tile([128, 2], F32, tag="T2")
                    R2 = work.tile([128, 2], F32, tag="R2")
                    quake_rsqrt(V2, Yi2, T2, R2, 2)
                    R2v = R2.rearrange("p (s o) -> p s o", s=2)
                    lnc = work.tile([128, BPC, 4], F32, tag="lnc")
                    nc.vector.tensor_tensor(
                        lnc, cv[:, :, 0], cmean.to_broadcast((128, 2, 4)), OP.subtract
                    )
                    nc.vector.tensor_tensor(
                        lnc, lnc, R2v.to_broadcast((128, 2, 4)), OP.mult
                    )
                    if apply_ln_affine:
                        lnwv = lnw[:, l].rearrange("p (g cc) -> p g cc", g=4)
                        lnbv = lnb[:, l].rearrange("p (g cc) -> p g cc", g=4)
                        nc.vector.tensor_tensor(
                            lnc, lnc, lnwv[:, 0:1].to_broadcast((128, 2, 4)), OP.mult
                        )
                        nc.vector.tensor_tensor(
                            lnc, lnc, lnbv[:, 0:1].to_broadcast((128, 2, 4)), OP.add
                        )
                    nc.scalar.activation(lnc, lnc, AF.Tanh)
                    # --- h = o * tanh(ln(c)) ---
                    hdst = h0t[:, :, :] if l == 0 else ys_sb[:, :, t]
                    nc.vector.tensor_tensor(hdst, wk[:, :, 2], lnc, OP.mult)

            # ---- output DMA ----
            for s in range(BPC):
                nc.sync.dma_start(out=ys_d[:, s], in_=ys_sb[:, s].bitcast(F32))

    nc.compile()
    return nc


_CACHE = {}


def _get_program(s_steps=S, affine=False):
    key = (s_steps, affine)
    if key not in _CACHE:
        _CACHE[key] = build_program(s_steps, apply_ln_affine=affine)
    return _CACHE[key]


def make_in_maps(x, wih0, whh0, bih0, bhh0, wih1, whh1, bih1, bhh1, ln_w, ln_b, s_steps=S):
    """Host-side preprocessing: shard + reformat inputs for the 8 cores."""
    x = np.asarray(x, np.float32)[:, :s_steps]
    perm = GATE_PERM
    in_maps = []
    for c in range(NCORES):
        sl = slice(c * BPC, (c + 1) * BPC)
        xs = x[sl]  # [BPC, s, 512]
        w0p = np.asarray(wih0, np.float32)[sl][:, perm]  # [BPC, 2048, 512]
        wh0p = np.asarray(whh0, np.float32)[sl][:, perm]
        w1p = (np.asarray(wih1, np.float32) + np.asarray(whh1, np.float32))[sl][:, perm]
        b0p = (np.asarray(bih0, np.float32) + np.asarray(bhh0, np.float32))[sl][:, perm]
        b1p = (np.asarray(bih1, np.float32) + np.asarray(bhh1, np.float32))[sl][:, perm]

        # position convention: vector index pos maps to (p = pos//4, cc = pos%4);
        # contraction block k: h-tile column k holds h[4p + k]
        # xT[s, k, d', t] = x[s, t, 4d'+k]
        xT = np.ascontiguousarray(
            xs.transpose(0, 2, 1).reshape(BPC, 128, 4, s_steps).transpose(0, 2, 1, 3)
        )
        # w0T[s, m=(g,q), k, d', j] = W0_perm[512g + 4j + q, 4d' + k]
        w0v = w0p.reshape(BPC, 4, 128, 4, 128, 4)  # [s, g, j, q, d', k]
        w0T = np.ascontiguousarray(w0v.transpose(0, 1, 3, 5, 4, 2).reshape(BPC, 16, 4, 128, 128))
        # whT[s, l, k, d', n]: col group g streams n in [512g, 512g+512);
        # streamed col j of group g -> W row 512g + 4*(j%128) + j//128
        n_idx = np.arange(2048)
        gg, j = n_idx // 512, n_idx % 512
        r_idx = 512 * gg + 4 * (j % 128) + (j // 128)
        whT = np.stack([wh0p, w1p], axis=1)[:, :, r_idx]  # [BPC, 2, 2048, 512]
        whT = np.ascontiguousarray(
            whT.reshape(BPC, 2, 2048, 128, 4).transpose(0, 1, 4, 3, 2)
        )
        # b0col[p, s, g*4+cc] = b0_perm[512g + 4p + cc]
        b0col = np.ascontiguousarray(
            b0p.reshape(BPC, 4, 128, 4).transpose(2, 0, 1, 3).reshape(128, BPC, 16)
        )
        b1col = np.ascontiguousarray(
            b1p.reshape(BPC, 4, 128, 4).transpose(2, 0, 1, 3).reshape(128, BPC, 16)
        )
        lnw_rep = np.ascontiguousarray(
            np.broadcast_to(
                np.asarray(ln_w, np.float32).reshape(2, 128, 4).transpose(1, 0, 2)[:, :, None, :],
                (128, 2, 4, 4),
            ).reshape(128, 2, 16)
        )
        lnb_rep = np.ascontiguousarray(
            np.broadcast_to(
                np.asarray(ln_b, np.float32).reshape(2, 128, 4).transpose(1, 0, 2)[:, :, None, :],
                (128, 2, 4, 4),
            ).reshape(128, 2, 16)
        )
        in_maps.append(
            {
                "xT": xT,
                "w0T": w0T,
                "whT": whT,
                "b0col": b0col,
                "b1col": b1col,
                "lnw": lnw_rep,
                "lnb": lnb_rep,
            }
        )
    return in_maps


def assemble_output(results, s_steps=S):
    ys = np.empty((B, s_steps, H), np.float32)
    for c in range(NCORES):
        out = results[c]["ys"]  # [128, BPC, s, 4]
        for s in range(BPC):
            # ys[b, t, 4p+cc] = out[p, s, t, cc]
            ys[c * BPC + s] = out[:, s].transpose(1, 0, 2).reshape(s_steps, H)
    return ys


def kernel(**inputs):
    s_steps = S
    affine = not (
        np.all(np.asarray(inputs["ln_w"]) == 1.0)
        and np.all(np.asarray(inputs["ln_b"]) == 0.0)
    )
    nc = _get_program(s_steps, affine)
    in_maps = make_in_maps(**inputs, s_steps=s_steps)
    res = run_bass_kernel_spmd(nc, in_maps, list(range(NCORES)))
    return assemble_output(res.results, s_steps)


if __name__ == "__main__":
    # quick small-S self-test against a numpy reference
    s_steps = int(sys.argv[1]) if len(sys.argv) > 1 else 8

    rng = np.random.default_rng(0)
    WS = 0.02
    inputs = {
        "x": rng.standard_normal((B, S, D), np.float32),
        "wih0": rng.standard_normal((B, 2048, D), np.float32) * WS,
        "whh0": rng.standard_normal((B, 2048, H), np.float32) * WS,
        "bih0": rng.standard_normal((B, 2048), np.float32) * WS,
        "bhh0": rng.standard_normal((B, 2048), np.float32) * WS,
        "wih1": rng.standard_normal((B, 2048, H), np.float32) * WS,
        "whh1": rng.standard_normal((B, 2048, H), np.float32) * WS,
        "bih1": rng.standard_normal((B, 2048), np.float32) * WS,
        "bhh1": rng.standard_normal((B, 2048), np.float32) * WS,
        "ln_w": np.ones((2, H), np.float32),
        "ln_b": np.zeros((2, H), np.float32),
    }

    def np_ref(inputs, s_steps):
        def ln(v):
            m = v.mean(-1, keepdims=True)
            va = ((v - m) ** 2).mean(-1, keepdims=True)
            return (v - m) / np.sqrt(va + EPS)

        def sig(v):
            return 1.0 / (1.0 + np.exp(-v))

        x = inputs["x"][:, :s_steps].astype(np.float64)
        h = np.zeros((B, H))
        c = np.zeros((B, H))
        ys = np.zeros((B, s_steps, H))
        for t in range(s_steps):
            cur = x[:, t]
            for l, (wi, wh, bi, bh) in enumerate(
                [
                    (inputs["wih0"], inputs["whh0"], inputs["bih0"], inputs["bhh0"]),
                    (inputs["wih1"], inputs["whh1"], inputs["bih1"], inputs["bhh1"]),
                ]
            ):
                gates = (
                    np.einsum("bgd,bd->bg", wi.astype(np.float64), cur)
                    + np.einsum("bgh,bh->bg", wh.astype(np.float64), h)
                    + bi
                    + bh
                )
                i, f, g, o = np.split(gates, 4, axis=1)
                i, f, g, o = sig(ln(i)), sig(ln(f)), np.tanh(ln(g)), sig(ln(o))
                c = f * c + i * g
                h = o * np.tanh(ln(c))
                cur = h
            ys[:, t] = h
        return ys

    import time

    t0 = time.time()
    nc = build_program(s_steps)
    print(f"build+schedule+compile: {time.time()-t0:.1f}s", flush=True)
    in_maps = make_in_maps(**inputs, s_steps=s_steps)
    t1 = time.time()
    res = run_bass_kernel_spmd(nc, in_maps, list(range(NCORES)))
    print(f"neff+run: {time.time()-t1:.1f}s", flush=True)
    got = assemble_output(res.results, s_steps)
    want = np_ref(inputs, s_steps)
    rel = np.abs(got - want).max() / max(np.abs(want).max(), 1e-9)
    print(f"S={s_steps}  max|want|={np.abs(want).max():.4f}  rel_err={rel:.3e}", flush=True)


def build_timed_runner(nc, in_maps):
    """Device-resident executor for timing: stages inputs once, returns a
    callable that runs the NEFF across the 8 cores and blocks."""
    import jax
    import numpy as np
    from jax.sharding import Mesh, PartitionSpec, NamedSharding
    from jax.experimental.shard_map import shard_map
    from concourse import bass2jax, mybir as _mb
    from concourse.bass2jax import _bass_exec_p, partition_id_tensor, install_neuronx_cc_hook

    install_neuronx_cc_hook()
    n_cores = len(in_maps)
    part_name = nc.partition_id_tensor.name if nc.partition_id_tensor else None
    in_names, out_names, out_avals, zero_outs = [], [], [], []
    for alloc in nc.m.functions[0].allocations:
        if not isinstance(alloc, _mb.MemoryLocationSet):
            continue
        name = alloc.memorylocations[0].name
        if alloc.kind == "ExternalInput":
            if name != part_name:
                in_names.append(name)
        elif alloc.kind == "ExternalOutput":
            out_names.append(name)
            shape = tuple(alloc.tensor_shape)
            dtype = _mb.dt.np(alloc.dtype)
            out_avals.append(jax.core.ShapedArray(shape, dtype))
            zero_outs.append(np.zeros(shape, dtype))
    n_params = len(in_names)
    all_names = in_names + out_names
    if part_name is not None:
        all_names = all_names + [part_name]

    def _body(*args):
        operands = list(args)
        if part_name is not None:
            operands.append(partition_id_tensor())
        outs = _bass_exec_p.bind(
            *operands,
            out_avals=tuple(out_avals),
            in_names=tuple(all_names),
            out_names=tuple(out_names),
            lowering_input_output_aliases=(),
            sim_require_finite=True,
            sim_require_nnan=True,
            nc=nc,
        )
        return tuple(outs)

    devices = jax.devices()[:n_cores]
    mesh = Mesh(np.asarray(devices), ("core",))
    in_specs = (PartitionSpec("core"),) * (n_params + len(out_names))
    out_specs = (PartitionSpec("core"),) * len(out_names)
    sharded = jax.jit(
        shard_map(_body, mesh=mesh, in_specs=in_specs, out_specs=out_specs, check_rep=False),
        keep_unused=True,
    )
    sh = NamedSharding(mesh, PartitionSpec("core"))
    concat_in = [
        jax.device_put(
            np.concatenate(
                [np.asarray(in_maps[c][k]) for c in range(n_cores)], axis=0
            ),
            sh,
        )
        for k in in_names
    ]
    concat_zeros = [
        jax.device_put(np.zeros((n_cores * z.shape[0], *z.shape[1:]), z.dtype), sh)
        for z in zero_outs
    ]

    def run():
        outs = sharded(*concat_in, *concat_zeros)
        jax.block_until_ready(outs)
        return outs

    return run
